# revision 5
# baseline (speedup 1.0000x reference)
# BitConvBlock Trainium2 kernel: LayerNorm -> activation int8-quant ->
# ternary weight quant -> conv1d(K=3, pad 1) -> rescale.
#
# Sharding: data-parallel over batch (B=8) across the 8 NeuronCores; every
# core gets one batch element plus replicated W / ln params, computes its
# full [T, C] output slice, host stacks the results.
#
# Exactness strategy: after quantization x_q is an integer in [-127, 127]
# and w_q is in {-1, 0, 1}; both are exact in bf16 and every partial sum is
# < 2^24, so bf16 matmuls with fp32 PSUM accumulation reproduce the fp32
# reference conv bit-exactly. Rounding uses the fp32 +-1.5*2^23 trick which
# is round-to-nearest-even, matching jnp.round.

import numpy as np

import concourse.bacc as bacc
import concourse.bass as bass
import concourse.mybir as mybir
import concourse.tile as tile
from concourse.bass_utils import run_bass_kernel_spmd
from concourse.masks import make_identity

F32 = mybir.dt.float32
BF16 = mybir.dt.bfloat16
AX = mybir.AxisListType
OP = mybir.AluOpType
AF = mybir.ActivationFunctionType

QP = 127.0
EPS_LN = 1e-5
EPS_CLAMP = 1e-5
RC = 1.5 * 2.0**23  # fp32 round-to-nearest-even magic constant
N_CORES = 8
KW = 3  # conv kernel width


def bcast_ap(ap, nparts):
    """Partition-broadcast a 1-D AP (stride-0 partition dim) for SWDGE DMA."""
    return bass.AP(tensor=ap.tensor, offset=ap.offset, ap=[[0, nparts]] + list(ap.ap))


def build_kernel(T, C, beta_zero, n_cores=N_CORES):
    """Build and compile the per-core Bass program for x:[T,C] W:[C,C,3]."""
    assert T % 128 == 0 and C % 128 == 0
    NT = T // 128            # time tiles
    NCC = C // 128           # channel chunks of 128
    OSL = min(512, C)        # output-channel slab (one PSUM bank)
    NH = C // OSL            # slabs per tile
    TQ = min(1024, T)        # transpose granularity along T
    NQ = T // TQ
    NTQ = TQ // 128          # time tiles per transpose chunk
    SUB = min(512, C)        # bn_stats subgroup
    NS = C // SUB
    XPAD = 16                # left pad in xqT so xbar writes stay 32B-aligned
    W_COUNT = float(C * C * KW)

    nc = bacc.Bacc("TRN2", target_bir_lowering=False, debug=False,
                   num_devices=n_cores)
    x_d = nc.dram_tensor("x", [T, C], F32, kind="ExternalInput")
    g_d = nc.dram_tensor("ln_gamma", [C], F32, kind="ExternalInput")
    b_d = nc.dram_tensor("ln_beta", [C], F32, kind="ExternalInput")
    w_d = nc.dram_tensor("W", [C, C, KW], F32, kind="ExternalInput")
    out_d = nc.dram_tensor("out", [T, C], F32, kind="ExternalOutput")

    with tile.TileContext(nc) as tc:
        import contextlib
        with contextlib.ExitStack() as ctx:
            dram = ctx.enter_context(tc.tile_pool(name="dram", bufs=1, space="DRAM"))
            xq_dram = dram.tile([T, C], BF16)
            wq_dram = dram.tile([KW, C, C], BF16)
            rows_dram = dram.tile([3, C, 1], F32)  # A, B, r gathered rows

            const = ctx.enter_context(tc.tile_pool(name="const", bufs=1))
            ident = const.tile([128, 128], F32)
            make_identity(nc, ident[:])
            ones_col = const.tile([128, 1], F32)
            nc.vector.memset(ones_col[:], 1.0)
            ones_row = const.tile([1, 128], F32)
            nc.vector.memset(ones_row[:], 1.0)
            c127 = const.tile([128, 1], F32)
            nc.vector.memset(c127[:], QP)

            rsig_all = const.tile([128, NT], F32)
            nmr_all = const.tile([128, NT], F32)   # -mu * rsig
            wabs = const.tile([128, NCC], F32)
            beta_col = const.tile([128, 1], F32)
            binv_col = const.tile([128, 1], F32)

            # chain accumulators (split across DVE and GPSIMD)
            accs = [const.tile([128, C], F32, tag=f"acc{i}", name=f"acc{i}")
                    for i in range(4)]

            A_b = const.tile([128, C], F32)
            B_b = const.tile([128, C], F32)
            r_b = const.tile([128, C], F32)

            # big persistent bf16 operands
            xqt_all = const.tile([128, NCC, T + 2 * XPAD], BF16)
            wqt_all = const.tile([128, KW, NCC, C], BF16)

            xin = ctx.enter_context(tc.tile_pool(name="xin", bufs=2))
            xhat_p = ctx.enter_context(tc.tile_pool(name="xhat", bufs=2))
            xq_p = ctx.enter_context(tc.tile_pool(name="xq", bufs=3))
            win_p = ctx.enter_context(tc.tile_pool(name="win", bufs=1))
            wq_p = ctx.enter_context(tc.tile_pool(name="wq", bufs=4))
            yout = ctx.enter_context(tc.tile_pool(name="yout", bufs=4))
            small = ctx.enter_context(tc.tile_pool(name="small", bufs=10))
            st_p = ctx.enter_context(tc.tile_pool(name="st", bufs=2))

            psum_mm = ctx.enter_context(
                tc.tile_pool(name="psum_mm", bufs=6, space="PSUM"))
            psum_ms = ctx.enter_context(
                tc.tile_pool(name="psum_ms", bufs=2, space="PSUM"))

            def ptile():
                return psum_ms.tile([128, 128], F32, tag="ms", name="pms")

            # ---------------- Pass X1: LN stats + channel extrema --------
            # (W abs-sum pass interleaved with the first NCC x-tiles)
            for it in range(NT):
                xt = xin.tile([128, C], F32)
                nc.sync.dma_start(out=xt[:], in_=x_d[it * 128:(it + 1) * 128, :])

                st6 = st_p.tile([128, NS, 6], F32)
                for s in range(NS):
                    nc.vector.bn_stats(st6[:, s, :], xt[:, s * SUB:(s + 1) * SUB])
                mv = small.tile([128, 2], F32, tag="mv")
                nc.vector.bn_aggr(mv[:], st6[:])

                # rsig = rsqrt(var + eps) with one Newton step on top of the
                # ACT sqrt spline + DVE reciprocal.
                ve = small.tile([128, 1], F32, tag="ve")
                nc.vector.tensor_scalar_add(ve[:], mv[:, 1:2], EPS_LN)
                s0 = small.tile([128, 1], F32, tag="s0")
                nc.scalar.activation(s0[:], ve[:], AF.Sqrt)
                r0 = small.tile([128, 1], F32, tag="r0")
                nc.vector.reciprocal(r0[:], s0[:])
                r2 = small.tile([128, 1], F32, tag="r2")
                nc.vector.tensor_mul(r2[:], r0[:], r0[:])
                vr2 = small.tile([128, 1], F32, tag="vr2")
                nc.vector.tensor_mul(vr2[:], r2[:], ve[:])
                h = small.tile([128, 1], F32, tag="h")
                nc.vector.tensor_scalar(h[:], vr2[:], -0.5, 1.5, op0=OP.mult,
                                        op1=OP.add)
                nc.vector.tensor_tensor(rsig_all[:, it:it + 1], r0[:], h[:],
                                        op=OP.mult)
                mr = small.tile([128, 1], F32, tag="mr")
                nc.vector.tensor_tensor(mr[:], mv[:, 0:1],
                                        rsig_all[:, it:it + 1], op=OP.mult)
                nc.vector.tensor_scalar_mul(nmr_all[:, it:it + 1], mr[:], -1.0)

                xh = xhat_p.tile([128, C], F32)
                nc.scalar.activation(xh[:], xt[:], AF.Identity,
                                     bias=nmr_all[:, it:it + 1],
                                     scale=rsig_all[:, it:it + 1])

                amx, amn = accs[it % 2], accs[2 + it % 2]
                if it < 2:
                    nc.vector.tensor_copy(amx[:], xh[:])
                    nc.gpsimd.tensor_copy(amn[:], xh[:])
                else:
                    nc.vector.tensor_tensor(amx[:], amx[:], xh[:], op=OP.max)
                    nc.vector.tensor_tensor(amn[:], amn[:], xh[:], op=OP.min)

                # ---- interleave W abs-sum (pass W1) ----
                if it < NCC:
                    ot = it
                    wt = win_p.tile([128, C, KW], F32)
                    nc.sync.dma_start(out=wt[:],
                                      in_=w_d[ot * 128:(ot + 1) * 128, :, :])
                    nc.vector.tensor_reduce(wabs[:, ot:ot + 1], wt[:], axis=AX.XY,
                                            op=OP.add, apply_absolute_value=True)

            # combine the split chain accumulators
            nc.vector.tensor_tensor(accs[0][:], accs[0][:], accs[1][:],
                                    op=OP.max)
            nc.vector.tensor_tensor(accs[2][:], accs[2][:], accs[3][:],
                                    op=OP.min)

            # ---------------- beta_w = max(mean|W|, eps) ------------------
            wsum = small.tile([128, 1], F32, tag="wsum")
            nc.vector.reduce_sum(wsum[:], wabs[:], axis=AX.X)
            ps1 = psum_ms.tile([1, 1], F32, tag="ms")
            nc.tensor.matmul(ps1[:], ones_col[:], wsum[:], start=True, stop=True)
            bsc = small.tile([1, 1], F32, tag="bsc")
            nc.vector.tensor_scalar(bsc[:], ps1[:], 1.0 / W_COUNT, EPS_CLAMP,
                                    op0=OP.mult, op1=OP.max)
            psb = psum_ms.tile([128, 1], F32, tag="ms")
            nc.tensor.matmul(psb[:], ones_row[:], bsc[:], start=True, stop=True)
            nc.vector.tensor_copy(beta_col[:], psb[:])
            nc.vector.reciprocal(binv_col[:], beta_col[:])

            # ---------------- per-channel scales --------------------------
            for j in range(NCC):
                cs = slice(j * 128, (j + 1) * 128)
                g_col = small.tile([128, 1], F32, tag="gcol")
                nc.sync.dma_start(out=g_col[:], in_=g_d[cs].rearrange("(c o) -> c o", o=1))
                pmx = ptile()
                nc.tensor.transpose(pmx[:], accs[0][:, cs], ident[:])
                mxc = small.tile([128, 1], F32, tag="mxc")
                nc.vector.tensor_reduce(mxc[:], pmx[:], axis=AX.X, op=OP.max)
                pmn = ptile()
                nc.tensor.transpose(pmn[:], accs[2][:, cs], ident[:])
                mnc = small.tile([128, 1], F32, tag="mnc")
                nc.vector.tensor_reduce(mnc[:], pmn[:], axis=AX.X, op=OP.min)
                # endpoints of the per-channel affine map, then amax =
                # max(t1, t2, -t1, -t2) (abs via negate+max)
                t1 = small.tile([128, 1], F32, tag="t1")
                t2 = small.tile([128, 1], F32, tag="t2")
                if beta_zero:
                    nc.vector.tensor_tensor(t1[:], g_col[:], mxc[:], op=OP.mult)
                    nc.vector.tensor_tensor(t2[:], g_col[:], mnc[:], op=OP.mult)
                else:
                    b_col = small.tile([128, 1], F32, tag="bcol")
                    nc.sync.dma_start(out=b_col[:],
                                      in_=b_d[cs].rearrange("(c o) -> c o", o=1))
                    nc.vector.tensor_scalar(t1[:], mxc[:], g_col[:], b_col[:],
                                            op0=OP.mult, op1=OP.add)
                    nc.vector.tensor_scalar(t2[:], mnc[:], g_col[:], b_col[:],
                                            op0=OP.mult, op1=OP.add)
                m1 = small.tile([128, 1], F32, tag="m1")
                nc.vector.tensor_tensor(m1[:], t1[:], t2[:], op=OP.max)
                n1 = small.tile([128, 1], F32, tag="n1")
                nc.vector.tensor_tensor(n1[:], t1[:], t2[:], op=OP.min)
                amax = small.tile([128, 1], F32, tag="amax")
                nc.vector.tensor_scalar(amax[:], n1[:], -1.0, m1[:],
                                        op0=OP.mult, op1=OP.max)

                gq = small.tile([128, 1], F32, tag="gq")
                nc.vector.tensor_scalar_max(gq[:], amax[:], EPS_CLAMP)
                ginv = small.tile([128, 1], F32, tag="ginv")
                nc.vector.reciprocal(ginv[:], gq[:])
                sc = small.tile([128, 1], F32, tag="sc")
                nc.vector.tensor_scalar_mul(sc[:], ginv[:], QP)
                scinv = small.tile([128, 1], F32, tag="scinv")
                nc.vector.reciprocal(scinv[:], sc[:])
                r_col = small.tile([128, 1], F32, tag="rcol")
                nc.vector.tensor_tensor(r_col[:], beta_col[:], scinv[:],
                                        op=OP.mult)
                A_col = small.tile([128, 1], F32, tag="Acol")
                nc.vector.tensor_tensor(A_col[:], g_col[:], sc[:], op=OP.mult)
                nc.scalar.dma_start(out=rows_dram[0, cs, :], in_=A_col[:])
                nc.scalar.dma_start(out=rows_dram[2, cs, :], in_=r_col[:])
                if not beta_zero:
                    B_col = small.tile([128, 1], F32, tag="Bcol")
                    nc.vector.tensor_tensor(B_col[:], b_col[:], sc[:], op=OP.mult)
                    nc.scalar.dma_start(out=rows_dram[1, cs, :], in_=B_col[:])

            # broadcast rows across partitions
            nc.gpsimd.dma_start(out=A_b[:], in_=bcast_ap(rows_dram[0, :, 0], 128))
            nc.gpsimd.dma_start(out=r_b[:], in_=bcast_ap(rows_dram[2, :, 0], 128))
            if not beta_zero:
                nc.gpsimd.dma_start(out=B_b[:],
                                    in_=bcast_ap(rows_dram[1, :, 0], 128))

            # ---------------- Pass W2: quantize weights -------------------
            for ot in range(NCC):
                wt = win_p.tile([128, C, KW], F32)
                nc.sync.dma_start(out=wt[:], in_=w_d[ot * 128:(ot + 1) * 128, :, :])
                nc.vector.tensor_scalar(wt[:], wt[:], binv_col[:], 1.0,
                                        op0=OP.mult, op1=OP.min)
                nc.gpsimd.tensor_scalar(wt[:], wt[:], -1.0, RC,
                                        op0=OP.max, op1=OP.add)
                for k in range(KW):
                    wqk = wq_p.tile([128, C], BF16)
                    nc.vector.tensor_scalar_add(wqk[:], wt[:, :, k], -RC)
                    nc.scalar.dma_start(
                        out=wq_dram[k, ot * 128:(ot + 1) * 128, :], in_=wqk[:])
            for k in range(KW):
                for j in range(NCC):
                    nc.sync.dma_start_transpose(
                        wqt_all[:, k, j, :], wq_dram[k, :, j * 128:(j + 1) * 128])

            # ---------------- Pass X2 + transpose + matmul ----------------
            for j in range(NCC):
                nc.vector.memset(xqt_all[:, j, XPAD - 1:XPAD], 0.0)
                nc.vector.memset(xqt_all[:, j, XPAD + T:XPAD + T + 1], 0.0)

            for q in range(NQ):
                for itq in range(NTQ):
                    it = q * NTQ + itq
                    xt = xin.tile([128, C], F32)
                    nc.sync.dma_start(out=xt[:],
                                      in_=x_d[it * 128:(it + 1) * 128, :])
                    xh = xhat_p.tile([128, C], F32)
                    nc.scalar.activation(xh[:], xt[:], AF.Identity,
                                         bias=nmr_all[:, it:it + 1],
                                         scale=rsig_all[:, it:it + 1])
                    nc.vector.tensor_tensor(xh[:], xh[:], A_b[:], op=OP.mult)
                    if not beta_zero:
                        nc.vector.tensor_tensor(xh[:], xh[:], B_b[:], op=OP.add)
                    xq = xq_p.tile([128, C], BF16)
                    nc.vector.tensor_scalar(xq[:], xh[:], RC, -RC,
                                            op0=OP.add, op1=OP.add)
                    nc.scalar.dma_start(out=xq_dram[it * 128:(it + 1) * 128, :],
                                        in_=xq[:])
                for j in range(NCC):
                    nc.sync.dma_start_transpose(
                        xqt_all[:, j, XPAD + q * TQ:XPAD + (q + 1) * TQ],
                        xq_dram[q * TQ:(q + 1) * TQ, j * 128:(j + 1) * 128])

                for itq in range(NTQ):
                    it = q * NTQ + itq
                    pss = [psum_mm.tile([128, OSL], F32, tag="mm", name="pmm")
                           for _ in range(NH)]
                    for j in range(NCC):
                        for k in range(KW):
                            lhsT = xqt_all[:, j, XPAD + it * 128 + k - 1:
                                           XPAD + it * 128 + k - 1 + 128]
                            first = (j == 0 and k == 0)
                            last = (j == NCC - 1 and k == KW - 1)
                            for h in range(NH):
                                nc.tensor.matmul(
                                    pss[h][:], lhsT,
                                    wqt_all[:, k, j, h * OSL:(h + 1) * OSL],
                                    start=first, stop=last)
                    for h in range(NH):
                        yt = yout.tile([128, OSL], F32)
                        nc.vector.tensor_tensor(
                            yt[:], pss[h][:], r_b[:, h * OSL:(h + 1) * OSL],
                            op=OP.mult)
                        nc.scalar.dma_start(
                            out=out_d[it * 128:(it + 1) * 128,
                                      h * OSL:(h + 1) * OSL],
                            in_=yt[:])

    nc.compile()
    return nc


_NC_CACHE = {}


def _get_nc(T, C, beta_zero):
    key = (T, C, beta_zero)
    if key not in _NC_CACHE:
        _NC_CACHE[key] = build_kernel(T, C, beta_zero)
    return _NC_CACHE[key]


def run(inputs, trace=False):
    """Run the SPMD kernel; returns (output [B,T,C], BassKernelResults)."""
    x = np.ascontiguousarray(np.asarray(inputs["x"], dtype=np.float32))
    g = np.ascontiguousarray(np.asarray(inputs["ln_gamma"], dtype=np.float32))
    b = np.ascontiguousarray(np.asarray(inputs["ln_beta"], dtype=np.float32))
    W = np.ascontiguousarray(np.asarray(inputs["W"], dtype=np.float32))
    B, T, C = x.shape
    assert B == N_CORES, f"expected batch {N_CORES}, got {B}"
    beta_zero = bool(np.all(b == 0.0))
    nc = _get_nc(T, C, beta_zero)
    in_maps = [
        {"x": np.ascontiguousarray(x[i]), "ln_gamma": g, "ln_beta": b, "W": W}
        for i in range(B)
    ]
    res = run_bass_kernel_spmd(nc, in_maps, core_ids=list(range(N_CORES)),
                               trace=trace)
    out = np.stack([res.results[i]["out"] for i in range(B)], axis=0)
    return out, res


def kernel(**inputs) -> np.ndarray:
    out, _ = run(inputs)
    return out


# revision 6
# speedup vs baseline: 1.4616x; 1.4616x over previous
# BitConvBlock Trainium2 kernel: LayerNorm -> activation int8-quant ->
# ternary weight quant -> conv1d(K=3, pad 1) -> rescale.
#
# Sharding: data-parallel over batch (B=8) across the 8 NeuronCores; every
# core gets one batch element plus replicated W / ln params, computes its
# full [T, C] output slice, host stacks the results.
#
# Exactness strategy: after quantization x_q is an integer in [-127, 127]
# and w_q is in {-1, 0, 1}; both are exact in bf16 and every partial sum is
# < 2^24, so bf16 matmuls with fp32 PSUM accumulation reproduce the fp32
# reference conv bit-exactly. Rounding uses the fp32 +-1.5*2^23 trick which
# is round-to-nearest-even, matching jnp.round.

import numpy as np

import concourse.bacc as bacc
import concourse.bass as bass
import concourse.mybir as mybir
import concourse.tile as tile
from concourse.bass_utils import run_bass_kernel_spmd
from concourse.masks import make_identity

F32 = mybir.dt.float32
BF16 = mybir.dt.bfloat16
AX = mybir.AxisListType
OP = mybir.AluOpType
AF = mybir.ActivationFunctionType

QP = 127.0
EPS_LN = 1e-5
EPS_CLAMP = 1e-5
RC = 1.5 * 2.0**23  # fp32 round-to-nearest-even magic constant
N_CORES = 8
KW = 3  # conv kernel width


def bcast_ap(ap, nparts):
    """Partition-broadcast a 1-D AP (stride-0 partition dim) for SWDGE DMA."""
    return bass.AP(tensor=ap.tensor, offset=ap.offset, ap=[[0, nparts]] + list(ap.ap))


def build_kernel(T, C, beta_zero, n_cores=N_CORES):
    """Build and compile the per-core Bass program for x:[T,C] W:[C,C,3]."""
    assert T % 128 == 0 and C % 128 == 0
    NT = T // 128            # time tiles
    NCC = C // 128           # channel chunks of 128
    OSL = min(512, C)        # output-channel slab (one PSUM bank)
    NH = C // OSL            # slabs per tile
    TQ = min(1024, T)        # transpose granularity along T
    NQ = T // TQ
    NTQ = TQ // 128          # time tiles per transpose chunk
    SUB = min(512, C)        # bn_stats subgroup
    NS = C // SUB
    XPAD = 16                # left pad in xqT so xbar writes stay 32B-aligned
    W_COUNT = float(C * C * KW)

    nc = bacc.Bacc("TRN2", target_bir_lowering=False, debug=False,
                   num_devices=n_cores)
    x_d = nc.dram_tensor("x", [T, C], F32, kind="ExternalInput")
    g_d = nc.dram_tensor("ln_gamma", [C], F32, kind="ExternalInput")
    b_d = nc.dram_tensor("ln_beta", [C], F32, kind="ExternalInput")
    w_d = nc.dram_tensor("W", [C, C, KW], F32, kind="ExternalInput")
    out_d = nc.dram_tensor("out", [T, C], F32, kind="ExternalOutput")

    with tile.TileContext(nc) as tc:
        import contextlib
        with contextlib.ExitStack() as ctx:
            dram = ctx.enter_context(tc.tile_pool(name="dram", bufs=1, space="DRAM"))
            xq_dram = dram.tile([T, C], BF16)
            wq_dram = dram.tile([KW, C, C], BF16)
            rows_dram = dram.tile([3, C, 1], F32)  # A, B, r gathered rows

            const = ctx.enter_context(tc.tile_pool(name="const", bufs=1))
            ident = const.tile([128, 128], F32)
            make_identity(nc, ident[:])
            ones_col = const.tile([128, 1], F32)
            nc.vector.memset(ones_col[:], 1.0)
            ones_row = const.tile([1, 128], F32)
            nc.vector.memset(ones_row[:], 1.0)
            c127 = const.tile([128, 1], F32)
            nc.vector.memset(c127[:], QP)

            rsig_all = const.tile([128, NT], F32)
            nmr_all = const.tile([128, NT], F32)   # -mu * rsig
            wabs = const.tile([128, NCC], F32)
            beta_col = const.tile([128, 1], F32)
            binv_col = const.tile([128, 1], F32)

            # chain accumulators (split across DVE and GPSIMD)
            accs = [const.tile([128, C], F32, tag=f"acc{i}", name=f"acc{i}")
                    for i in range(4)]

            A_b = const.tile([128, C], F32)
            B_b = const.tile([128, C], F32)
            r_b = const.tile([128, C], F32)

            # big persistent bf16 operands
            xqt_all = const.tile([128, NCC, T + 2 * XPAD], BF16)
            wqt_all = const.tile([128, KW, NCC, C], BF16)

            xin = ctx.enter_context(tc.tile_pool(name="xin", bufs=2))
            xhat_p = ctx.enter_context(tc.tile_pool(name="xhat", bufs=2))
            xq_p = ctx.enter_context(tc.tile_pool(name="xq", bufs=3))
            win_p = ctx.enter_context(tc.tile_pool(name="win", bufs=1))
            wq_p = ctx.enter_context(tc.tile_pool(name="wq", bufs=4))
            yout = ctx.enter_context(tc.tile_pool(name="yout", bufs=4))
            small = ctx.enter_context(tc.tile_pool(name="small", bufs=10))
            st_p = ctx.enter_context(tc.tile_pool(name="st", bufs=2))

            psum_mm = ctx.enter_context(
                tc.tile_pool(name="psum_mm", bufs=6, space="PSUM"))
            psum_ms = ctx.enter_context(
                tc.tile_pool(name="psum_ms", bufs=2, space="PSUM"))

            def ptile():
                return psum_ms.tile([128, 128], F32, tag="ms", name="pms")

            # ---------------- Pass X1: LN stats + channel extrema --------
            # (W abs-sum pass interleaved with the first NCC x-tiles)
            for it in range(NT):
                xt = xin.tile([128, C], F32)
                nc.sync.dma_start(out=xt[:], in_=x_d[it * 128:(it + 1) * 128, :])

                st6 = st_p.tile([128, NS, 6], F32)
                for s in range(NS):
                    nc.vector.bn_stats(st6[:, s, :], xt[:, s * SUB:(s + 1) * SUB])
                mv = small.tile([128, 2], F32, tag="mv")
                nc.vector.bn_aggr(mv[:], st6[:])

                # rsig = rsqrt(var + eps) with one Newton step on top of the
                # ACT sqrt spline + DVE reciprocal.
                ve = small.tile([128, 1], F32, tag="ve")
                nc.vector.tensor_scalar_add(ve[:], mv[:, 1:2], EPS_LN)
                s0 = small.tile([128, 1], F32, tag="s0")
                nc.scalar.activation(s0[:], ve[:], AF.Sqrt)
                r0 = small.tile([128, 1], F32, tag="r0")
                nc.vector.reciprocal(r0[:], s0[:])
                r2 = small.tile([128, 1], F32, tag="r2")
                nc.vector.tensor_mul(r2[:], r0[:], r0[:])
                vr2 = small.tile([128, 1], F32, tag="vr2")
                nc.vector.tensor_mul(vr2[:], r2[:], ve[:])
                h = small.tile([128, 1], F32, tag="h")
                nc.vector.tensor_scalar(h[:], vr2[:], -0.5, 1.5, op0=OP.mult,
                                        op1=OP.add)
                nc.vector.tensor_tensor(rsig_all[:, it:it + 1], r0[:], h[:],
                                        op=OP.mult)
                mr = small.tile([128, 1], F32, tag="mr")
                nc.vector.tensor_tensor(mr[:], mv[:, 0:1],
                                        rsig_all[:, it:it + 1], op=OP.mult)
                nc.vector.tensor_scalar_mul(nmr_all[:, it:it + 1], mr[:], -1.0)

                xh = xhat_p.tile([128, C], F32)
                nc.scalar.activation(xh[:], xt[:], AF.Identity,
                                     bias=nmr_all[:, it:it + 1],
                                     scale=rsig_all[:, it:it + 1])

                amx, amn = accs[it % 2], accs[2 + it % 2]
                if it < 2:
                    nc.vector.tensor_copy(amx[:], xh[:])
                    nc.gpsimd.tensor_copy(amn[:], xh[:])
                else:
                    nc.vector.tensor_tensor(amx[:], amx[:], xh[:], op=OP.max)
                    nc.vector.tensor_tensor(amn[:], amn[:], xh[:], op=OP.min)

                # ---- interleave W abs-sum (pass W1) ----
                if it < NCC:
                    ot = it
                    wt = win_p.tile([128, C, KW], F32)
                    nc.sync.dma_start(out=wt[:],
                                      in_=w_d[ot * 128:(ot + 1) * 128, :, :])
                    nc.vector.tensor_reduce(wabs[:, ot:ot + 1], wt[:], axis=AX.XY,
                                            op=OP.add, apply_absolute_value=True)

            # combine the split chain accumulators
            nc.vector.tensor_tensor(accs[0][:], accs[0][:], accs[1][:],
                                    op=OP.max)
            nc.vector.tensor_tensor(accs[2][:], accs[2][:], accs[3][:],
                                    op=OP.min)

            # ---------------- beta_w = max(mean|W|, eps) ------------------
            wsum = small.tile([128, 1], F32, tag="wsum")
            nc.vector.reduce_sum(wsum[:], wabs[:], axis=AX.X)
            ps1 = psum_ms.tile([1, 1], F32, tag="ms")
            nc.tensor.matmul(ps1[:], ones_col[:], wsum[:], start=True, stop=True)
            bsc = small.tile([1, 1], F32, tag="bsc")
            nc.vector.tensor_scalar(bsc[:], ps1[:], 1.0 / W_COUNT, EPS_CLAMP,
                                    op0=OP.mult, op1=OP.max)
            psb = psum_ms.tile([128, 1], F32, tag="ms")
            nc.tensor.matmul(psb[:], ones_row[:], bsc[:], start=True, stop=True)
            nc.vector.tensor_copy(beta_col[:], psb[:])
            nc.vector.reciprocal(binv_col[:], beta_col[:])

            # ---------------- per-channel scales --------------------------
            for j in range(NCC):
                cs = slice(j * 128, (j + 1) * 128)
                g_col = small.tile([128, 1], F32, tag="gcol")
                nc.sync.dma_start(out=g_col[:], in_=g_d[cs].rearrange("(c o) -> c o", o=1))
                pmx = ptile()
                nc.tensor.transpose(pmx[:], accs[0][:, cs], ident[:])
                mxc = small.tile([128, 1], F32, tag="mxc")
                nc.vector.tensor_reduce(mxc[:], pmx[:], axis=AX.X, op=OP.max)
                pmn = ptile()
                nc.tensor.transpose(pmn[:], accs[2][:, cs], ident[:])
                mnc = small.tile([128, 1], F32, tag="mnc")
                nc.vector.tensor_reduce(mnc[:], pmn[:], axis=AX.X, op=OP.min)
                # endpoints of the per-channel affine map, then amax =
                # max(t1, t2, -t1, -t2) (abs via negate+max)
                t1 = small.tile([128, 1], F32, tag="t1")
                t2 = small.tile([128, 1], F32, tag="t2")
                if beta_zero:
                    nc.vector.tensor_tensor(t1[:], g_col[:], mxc[:], op=OP.mult)
                    nc.vector.tensor_tensor(t2[:], g_col[:], mnc[:], op=OP.mult)
                else:
                    b_col = small.tile([128, 1], F32, tag="bcol")
                    nc.sync.dma_start(out=b_col[:],
                                      in_=b_d[cs].rearrange("(c o) -> c o", o=1))
                    nc.vector.tensor_scalar(t1[:], mxc[:], g_col[:], b_col[:],
                                            op0=OP.mult, op1=OP.add)
                    nc.vector.tensor_scalar(t2[:], mnc[:], g_col[:], b_col[:],
                                            op0=OP.mult, op1=OP.add)
                m1 = small.tile([128, 1], F32, tag="m1")
                nc.vector.tensor_tensor(m1[:], t1[:], t2[:], op=OP.max)
                n1 = small.tile([128, 1], F32, tag="n1")
                nc.vector.tensor_tensor(n1[:], t1[:], t2[:], op=OP.min)
                amax = small.tile([128, 1], F32, tag="amax")
                nc.vector.tensor_scalar(amax[:], n1[:], -1.0, m1[:],
                                        op0=OP.mult, op1=OP.max)

                gq = small.tile([128, 1], F32, tag="gq")
                nc.vector.tensor_scalar_max(gq[:], amax[:], EPS_CLAMP)
                ginv = small.tile([128, 1], F32, tag="ginv")
                nc.vector.reciprocal(ginv[:], gq[:])
                sc = small.tile([128, 1], F32, tag="sc")
                nc.vector.tensor_scalar_mul(sc[:], ginv[:], QP)
                scinv = small.tile([128, 1], F32, tag="scinv")
                nc.vector.reciprocal(scinv[:], sc[:])
                r_col = small.tile([128, 1], F32, tag="rcol")
                nc.vector.tensor_tensor(r_col[:], beta_col[:], scinv[:],
                                        op=OP.mult)
                A_col = small.tile([128, 1], F32, tag="Acol")
                nc.vector.tensor_tensor(A_col[:], g_col[:], sc[:], op=OP.mult)
                nc.scalar.dma_start(out=rows_dram[0, cs, :], in_=A_col[:])
                nc.scalar.dma_start(out=rows_dram[2, cs, :], in_=r_col[:])
                if not beta_zero:
                    B_col = small.tile([128, 1], F32, tag="Bcol")
                    nc.vector.tensor_tensor(B_col[:], b_col[:], sc[:], op=OP.mult)
                    nc.scalar.dma_start(out=rows_dram[1, cs, :], in_=B_col[:])

            # broadcast rows across partitions
            nc.gpsimd.dma_start(out=A_b[:], in_=bcast_ap(rows_dram[0, :, 0], 128))
            nc.gpsimd.dma_start(out=r_b[:], in_=bcast_ap(rows_dram[2, :, 0], 128))
            if not beta_zero:
                nc.gpsimd.dma_start(out=B_b[:],
                                    in_=bcast_ap(rows_dram[1, :, 0], 128))

            # ---------------- Pass W2: quantize weights -------------------
            for ot in range(NCC):
                wt = win_p.tile([128, C, KW], F32)
                nc.sync.dma_start(out=wt[:], in_=w_d[ot * 128:(ot + 1) * 128, :, :])
                nc.vector.tensor_scalar(wt[:], wt[:], binv_col[:], 1.0,
                                        op0=OP.mult, op1=OP.min)
                nc.vector.tensor_scalar(wt[:], wt[:], -1.0, RC,
                                        op0=OP.max, op1=OP.add)
                for k in range(KW):
                    wqk = wq_p.tile([128, C], BF16)
                    nc.vector.tensor_scalar_add(wqk[:], wt[:, :, k], -RC)
                    nc.scalar.dma_start(
                        out=wq_dram[k, ot * 128:(ot + 1) * 128, :], in_=wqk[:])
            for k in range(KW):
                for j in range(NCC):
                    nc.sync.dma_start_transpose(
                        wqt_all[:, k, j, :], wq_dram[k, :, j * 128:(j + 1) * 128])

            # ---------------- Pass X2 + transpose + matmul ----------------
            for j in range(NCC):
                nc.vector.memset(xqt_all[:, j, XPAD - 1:XPAD], 0.0)
                nc.vector.memset(xqt_all[:, j, XPAD + T:XPAD + T + 1], 0.0)

            for q in range(NQ):
                for itq in range(NTQ):
                    it = q * NTQ + itq
                    xt = xin.tile([128, C], F32)
                    nc.sync.dma_start(out=xt[:],
                                      in_=x_d[it * 128:(it + 1) * 128, :])
                    xh = xhat_p.tile([128, C], F32)
                    nc.scalar.activation(xh[:], xt[:], AF.Identity,
                                         bias=nmr_all[:, it:it + 1],
                                         scale=rsig_all[:, it:it + 1])
                    nc.vector.tensor_tensor(xh[:], xh[:], A_b[:], op=OP.mult)
                    if not beta_zero:
                        nc.vector.tensor_tensor(xh[:], xh[:], B_b[:], op=OP.add)
                    xq = xq_p.tile([128, C], BF16)
                    nc.vector.tensor_scalar(xq[:], xh[:], RC, -RC,
                                            op0=OP.add, op1=OP.add)
                    nc.scalar.dma_start(out=xq_dram[it * 128:(it + 1) * 128, :],
                                        in_=xq[:])
                for j in range(NCC):
                    nc.sync.dma_start_transpose(
                        xqt_all[:, j, XPAD + q * TQ:XPAD + (q + 1) * TQ],
                        xq_dram[q * TQ:(q + 1) * TQ, j * 128:(j + 1) * 128])

                for itq in range(NTQ):
                    it = q * NTQ + itq
                    pss = [psum_mm.tile([128, OSL], F32, tag="mm", name="pmm")
                           for _ in range(NH)]
                    for j in range(NCC):
                        for k in range(KW):
                            lhsT = xqt_all[:, j, XPAD + it * 128 + k - 1:
                                           XPAD + it * 128 + k - 1 + 128]
                            first = (j == 0 and k == 0)
                            last = (j == NCC - 1 and k == KW - 1)
                            for h in range(NH):
                                nc.tensor.matmul(
                                    pss[h][:], lhsT,
                                    wqt_all[:, k, j, h * OSL:(h + 1) * OSL],
                                    start=first, stop=last)
                    for h in range(NH):
                        yt = yout.tile([128, OSL], F32)
                        nc.vector.tensor_tensor(
                            yt[:], pss[h][:], r_b[:, h * OSL:(h + 1) * OSL],
                            op=OP.mult)
                        nc.scalar.dma_start(
                            out=out_d[it * 128:(it + 1) * 128,
                                      h * OSL:(h + 1) * OSL],
                            in_=yt[:])

    nc.compile()
    return nc


_NC_CACHE = {}


def _get_nc(T, C, beta_zero):
    key = (T, C, beta_zero)
    if key not in _NC_CACHE:
        _NC_CACHE[key] = build_kernel(T, C, beta_zero)
    return _NC_CACHE[key]


def run(inputs, trace=False):
    """Run the SPMD kernel; returns (output [B,T,C], BassKernelResults)."""
    x = np.ascontiguousarray(np.asarray(inputs["x"], dtype=np.float32))
    g = np.ascontiguousarray(np.asarray(inputs["ln_gamma"], dtype=np.float32))
    b = np.ascontiguousarray(np.asarray(inputs["ln_beta"], dtype=np.float32))
    W = np.ascontiguousarray(np.asarray(inputs["W"], dtype=np.float32))
    B, T, C = x.shape
    assert B == N_CORES, f"expected batch {N_CORES}, got {B}"
    beta_zero = bool(np.all(b == 0.0))
    nc = _get_nc(T, C, beta_zero)
    in_maps = [
        {"x": np.ascontiguousarray(x[i]), "ln_gamma": g, "ln_beta": b, "W": W}
        for i in range(B)
    ]
    res = run_bass_kernel_spmd(nc, in_maps, core_ids=list(range(N_CORES)),
                               trace=trace)
    out = np.stack([res.results[i]["out"] for i in range(B)], axis=0)
    return out, res


def kernel(**inputs) -> np.ndarray:
    out, _ = run(inputs)
    return out


# revision 11
# speedup vs baseline: 1.5131x; 1.0352x over previous
# BitConvBlock Trainium2 kernel: LayerNorm -> activation int8-quant ->
# ternary weight quant -> conv1d(K=3, pad 1) -> rescale.
#
# Sharding: data-parallel over batch (B=8) across the 8 NeuronCores; every
# core gets one batch element plus replicated W / ln params, computes its
# full [T, C] output slice, host stacks the results.
#
# Exactness strategy: after quantization x_q is an integer in [-127, 127]
# and w_q is in {-1, 0, 1}; both are exact in bf16 and every partial sum is
# < 2^24, so bf16 matmuls with fp32 PSUM accumulation reproduce the fp32
# reference conv bit-exactly. Rounding uses the fp32 +-1.5*2^23 trick which
# is round-to-nearest-even, matching jnp.round (verified bit-exact on the
# ACT engine's FMA path).

import numpy as np

import concourse.bacc as bacc
import concourse.bass as bass
import concourse.mybir as mybir
import concourse.tile as tile
from concourse.bass_utils import run_bass_kernel_spmd
from concourse.masks import make_identity

F32 = mybir.dt.float32
BF16 = mybir.dt.bfloat16
AX = mybir.AxisListType
OP = mybir.AluOpType
AF = mybir.ActivationFunctionType

QP = 127.0
EPS_LN = 1e-5
EPS_CLAMP = 1e-5
RC = 1.5 * 2.0**23  # fp32 round-to-nearest-even magic constant
N_CORES = 8
KW = 3  # conv kernel width


def build_kernel(T, C, beta_zero, n_cores=N_CORES):
    """Build and compile the per-core Bass program for x:[T,C] W:[C,C,3]."""
    assert T % 128 == 0 and C % 128 == 0
    NT = T // 128            # time tiles
    NCC = C // 128           # channel chunks of 128
    OSL = min(512, C)        # output-channel slab (one PSUM bank)
    NH = C // OSL            # slabs per tile
    TQ = min(1024, T)        # transpose granularity along T
    NQ = T // TQ
    NTQ = TQ // 128          # time tiles per transpose chunk / stat group
    SUB = min(512, C)        # bn_stats subgroup
    NS = C // SUB
    XPAD = 16                # left pad in xqT so xbar writes stay 32B-aligned
    W_COUNT = float(C * C * KW)

    # distribute the weight-quantize o-tiles over groups 1..NQ-1
    if NQ > 1:
        per = -(-NCC // (NQ - 1))
        W2_SCHED = {g: list(range(per * (g - 1), min(per * g, NCC)))
                    for g in range(1, NQ)}
    else:
        W2_SCHED = {}
    W2_TAIL = [ot for ot in range(NCC)
               if not any(ot in v for v in W2_SCHED.values())]

    nc = bacc.Bacc("TRN2", target_bir_lowering=False, debug=False,
                   num_devices=n_cores)
    x_d = nc.dram_tensor("x", [T, C], F32, kind="ExternalInput")
    g_d = nc.dram_tensor("ln_gamma", [C], F32, kind="ExternalInput")
    b_d = nc.dram_tensor("ln_beta", [C], F32, kind="ExternalInput")
    w_d = nc.dram_tensor("W", [C, C, KW], F32, kind="ExternalInput")
    out_d = nc.dram_tensor("out", [T, C], F32, kind="ExternalOutput")

    with tile.TileContext(nc) as tc:
        import contextlib
        with contextlib.ExitStack() as ctx:
            dram = ctx.enter_context(tc.tile_pool(name="dram", bufs=1, space="DRAM"))
            xq_dram = dram.tile([T, C], BF16)

            const = ctx.enter_context(tc.tile_pool(name="const", bufs=1))
            ident = const.tile([128, 128], F32)
            make_identity(nc, ident[:])
            identb = const.tile([128, 128], BF16)
            nc.vector.tensor_copy(identb[:], ident[:])
            ones_col = const.tile([128, 1], F32)
            nc.vector.memset(ones_col[:], 1.0)
            ones_row = const.tile([1, 128], F32)
            nc.vector.memset(ones_row[:], 1.0)
            rcp_col = const.tile([128, 1], F32)
            nc.vector.memset(rcp_col[:], RC)
            rcn_col = const.tile([128, 1], F32)
            nc.vector.memset(rcn_col[:], -RC)

            mv_all = const.tile([128, NT, 2], F32)    # per-tile mean/var
            rsig_all = const.tile([128, NT], F32)
            nmr_all = const.tile([128, NT], F32)      # -mu * rsig
            wabs = const.tile([128, NCC], F32)
            beta_col = const.tile([128, 1], F32)
            binv_col = const.tile([128, 1], F32)

            amx_t = const.tile([128, C], F32)
            amn_t = const.tile([128, C], F32)

            A_b = const.tile([128, C], F32)
            B_b = const.tile([128, C], F32)
            r_b = const.tile([128, C], F32)

            # big persistent bf16 operands
            xqt_all = const.tile([128, NCC, T + 2 * XPAD], BF16)
            wqt_all = const.tile([128, KW, NCC, C], BF16)

            xin = ctx.enter_context(tc.tile_pool(name="xin", bufs=3))
            xin2 = ctx.enter_context(tc.tile_pool(name="xin2", bufs=3))
            xhat_p = ctx.enter_context(tc.tile_pool(name="xhat", bufs=2))
            xq_p = ctx.enter_context(tc.tile_pool(name="xq", bufs=2))
            win_p = ctx.enter_context(tc.tile_pool(name="win", bufs=1))
            wq_p = ctx.enter_context(tc.tile_pool(name="wq", bufs=1))
            yout = ctx.enter_context(tc.tile_pool(name="yout", bufs=3))
            small = ctx.enter_context(tc.tile_pool(name="small", bufs=4))
            st_p = ctx.enter_context(tc.tile_pool(name="st", bufs=2))
            grp_p = ctx.enter_context(tc.tile_pool(name="grp", bufs=3))

            psum_mm = ctx.enter_context(
                tc.tile_pool(name="psum_mm", bufs=6, space="PSUM"))
            psum_ms = ctx.enter_context(
                tc.tile_pool(name="psum_ms", bufs=2, space="PSUM"))

            def ptile():
                return psum_ms.tile([128, 512], F32, tag="ms", name="pms")

            def pbtile():
                return psum_ms.tile([128, 512], BF16, tag="ms", name="pmsb")

            def gtile():
                return grp_p.tile([128, NTQ], F32, tag="g1", name="gt")

            # ---- weight-quantize one o-tile + PE-transpose into wqt_all --
            def w2_tile(ot):
                wt = win_p.tile([128, C, KW], F32, tag="wt", name="wt")
                nc.gpsimd.dma_start(out=wt[:],
                                    in_=w_d[ot * 128:(ot + 1) * 128, :, :])
                # u = rne(w/beta) + RC via single-rounded FMA;
                # wq = Sign(u - RC)  (clip(round(v),-1,1) == sign(round(v)))
                nc.scalar.activation(wt[:], wt[:], AF.Identity,
                                     bias=rcp_col[:], scale=binv_col[:])
                wqs = wq_p.tile([128, C, KW], BF16, tag="wqs", name="wqs")
                nc.scalar.activation(wqs[:], wt[:], AF.Sign,
                                     bias=rcn_col[:], scale=1.0)
                # transpose [o,i] blocks -> wqt[k][i-chunk][:, ot*128: ...]
                for k in range(KW):
                    for jb2 in range(0, NCC, 4):
                        pb = pbtile()
                        nblk = min(4, NCC - jb2)
                        for b in range(nblk):
                            jb = jb2 + b
                            nc.tensor.transpose(
                                pb[:, b * 128:(b + 1) * 128],
                                wqs[:, jb * 128:(jb + 1) * 128, k],
                                identb[:])
                        nc.scalar.activation(
                            wqt_all[:, k, jb2:jb2 + nblk,
                                    ot * 128:(ot + 1) * 128],
                            pb[:, 0:nblk * 128], AF.Identity, bias=0.0,
                            scale=1.0)

            # ============ Pass X1: stats + extrema (grouped) ==============
            for g in range(NQ):
                for itq in range(NTQ):
                    it = g * NTQ + itq
                    xt = xin.tile([128, C], F32)
                    nc.sync.dma_start(out=xt[:],
                                      in_=x_d[it * 128:(it + 1) * 128, :])
                    st6 = st_p.tile([128, NS, 6], F32)
                    for sb in range(NS):
                        nc.vector.bn_stats(st6[:, sb, :],
                                           xt[:, sb * SUB:(sb + 1) * SUB])
                    nc.vector.bn_aggr(mv_all[:, it, :], st6[:])

                # batched rsqrt + one Newton step for the whole group
                gs = slice(g * NTQ, (g + 1) * NTQ)
                ve = gtile()
                nc.vector.tensor_scalar_add(ve[:], mv_all[:, gs, 1], EPS_LN)
                s0 = gtile()
                nc.scalar.activation(s0[:], ve[:], AF.Sqrt)
                r0 = gtile()
                nc.vector.reciprocal(r0[:], s0[:])
                r2 = gtile()
                nc.vector.tensor_mul(r2[:], r0[:], r0[:])
                nc.vector.tensor_mul(r2[:], r2[:], ve[:])
                nc.vector.tensor_scalar(r2[:], r2[:], -0.5, 1.5, op0=OP.mult,
                                        op1=OP.add)
                nc.vector.tensor_tensor(rsig_all[:, gs], r0[:], r2[:],
                                        op=OP.mult)
                mr = gtile()
                nc.vector.tensor_tensor(mr[:], mv_all[:, gs, 0],
                                        rsig_all[:, gs], op=OP.mult)
                nc.vector.tensor_scalar_mul(nmr_all[:, gs], mr[:], -1.0)

                # Pass X1b: xhat + extrema chains (x re-streamed, scalar q)
                for itq in range(NTQ):
                    it = g * NTQ + itq
                    xt = xin2.tile([128, C], F32, tag="xt2", name="xt2")
                    nc.scalar.dma_start(out=xt[:],
                                        in_=x_d[it * 128:(it + 1) * 128, :])
                    xh = xhat_p.tile([128, C], F32)
                    nc.scalar.activation(xh[:], xt[:], AF.Identity,
                                         bias=nmr_all[:, it:it + 1],
                                         scale=rsig_all[:, it:it + 1])
                    if it == 0:
                        nc.vector.tensor_copy(amx_t[:], xh[:])
                        nc.vector.tensor_copy(amn_t[:], xh[:])
                    else:
                        nc.vector.tensor_tensor(amx_t[:], amx_t[:], xh[:],
                                                op=OP.max)
                        nc.vector.tensor_tensor(amn_t[:], amn_t[:], xh[:],
                                                op=OP.min)

                if g == 0:
                    # W abs-sum on ACT (accum_out = free-axis sum)
                    for ot in range(NCC):
                        wt = win_p.tile([128, C, KW], F32, tag="wt", name="wt")
                        nc.gpsimd.dma_start(
                            out=wt[:], in_=w_d[ot * 128:(ot + 1) * 128, :, :])
                        nc.scalar.activation(wt[:], wt[:], AF.Abs,
                                             accum_out=wabs[:, ot:ot + 1])
                    # beta_w = max(mean|W|, eps); binv = 1/beta
                    wsum = small.tile([128, 1], F32, tag="wsum", name="wsum")
                    nc.vector.reduce_sum(wsum[:], wabs[:], axis=AX.X)
                    ps1 = psum_ms.tile([1, 1], F32, tag="ms", name="ps1")
                    nc.tensor.matmul(ps1[:], ones_col[:], wsum[:], start=True,
                                     stop=True)
                    bsc = small.tile([1, 1], F32, tag="bsc", name="bsc")
                    nc.vector.tensor_scalar(bsc[:], ps1[:], 1.0 / W_COUNT,
                                            EPS_CLAMP, op0=OP.mult, op1=OP.max)
                    psb = psum_ms.tile([128, 1], F32, tag="ms", name="psb")
                    nc.tensor.matmul(psb[:], ones_row[:], bsc[:], start=True,
                                     stop=True)
                    nc.vector.tensor_copy(beta_col[:], psb[:])
                    nc.vector.reciprocal(binv_col[:], beta_col[:])
                else:
                    for ot in W2_SCHED.get(g, []):
                        w2_tile(ot)
            for ot in W2_TAIL:
                w2_tile(ot)

            # ============ per-channel scales -> broadcast rows ============
            # gamma/beta reshaped [128, NCC]: element (p, j) = param[j*128+p]
            g_mat = small.tile([128, NCC], F32, tag="gmat", name="gmat")
            nc.gpsimd.dma_start(out=g_mat[:],
                                in_=g_d.ap().rearrange("(j p) -> p j", p=128))
            if not beta_zero:
                b_mat = small.tile([128, NCC], F32, tag="bmat", name="bmat")
                nc.gpsimd.dma_start(out=b_mat[:],
                                    in_=b_d.ap().rearrange("(j p) -> p j", p=128))
            Mx = small.tile([128, NCC], F32, tag="Mx", name="Mx")
            Mn = small.tile([128, NCC], F32, tag="Mn", name="Mn")
            for j in range(NCC):
                pmx = ptile()
                nc.tensor.transpose(pmx[:, 0:128],
                                    amx_t[:, j * 128:(j + 1) * 128], ident[:])
                nc.vector.tensor_reduce(Mx[:, j:j + 1], pmx[:, 0:128],
                                        axis=AX.X, op=OP.max)
                pmn = ptile()
                nc.tensor.transpose(pmn[:, 0:128],
                                    amn_t[:, j * 128:(j + 1) * 128], ident[:])
                nc.vector.tensor_reduce(Mn[:, j:j + 1], pmn[:, 0:128],
                                        axis=AX.X, op=OP.min)
            # batched endpoint math on [128, NCC]
            t1 = small.tile([128, NCC], F32, tag="t1", name="t1")
            t2 = small.tile([128, NCC], F32, tag="t2", name="t2")
            nc.vector.tensor_tensor(t1[:], g_mat[:], Mx[:], op=OP.mult)
            nc.vector.tensor_tensor(t2[:], g_mat[:], Mn[:], op=OP.mult)
            if not beta_zero:
                nc.vector.tensor_tensor(t1[:], t1[:], b_mat[:], op=OP.add)
                nc.vector.tensor_tensor(t2[:], t2[:], b_mat[:], op=OP.add)
            m1 = small.tile([128, NCC], F32, tag="m1", name="m1")
            nc.vector.tensor_tensor(m1[:], t1[:], t2[:], op=OP.max)
            nc.vector.tensor_scalar_mul(t2[:], t2[:], -1.0)
            nc.vector.tensor_scalar_mul(t1[:], t1[:], -1.0)
            nc.vector.tensor_tensor(m1[:], m1[:], t2[:], op=OP.max)
            nc.vector.tensor_tensor(m1[:], m1[:], t1[:], op=OP.max)  # amax
            nc.vector.tensor_scalar_max(m1[:], m1[:], EPS_CLAMP)     # gamma_q
            ginv = small.tile([128, NCC], F32, tag="ginv", name="ginv")
            nc.vector.reciprocal(ginv[:], m1[:])
            sc_m = small.tile([128, NCC], F32, tag="scm", name="scm")
            nc.vector.tensor_scalar_mul(sc_m[:], ginv[:], QP)
            scinv = small.tile([128, NCC], F32, tag="sci", name="sci")
            nc.vector.reciprocal(scinv[:], sc_m[:])
            A_m = small.tile([128, NCC], F32, tag="Am", name="Am")
            nc.vector.tensor_tensor(A_m[:], g_mat[:], sc_m[:], op=OP.mult)
            r_m = small.tile([128, NCC], F32, tag="rm", name="rm")
            nc.vector.tensor_scalar_mul(r_m[:], scinv[:], beta_col[:])
            if not beta_zero:
                B_m = small.tile([128, NCC], F32, tag="Bm", name="Bm")
                nc.vector.tensor_tensor(B_m[:], b_mat[:], sc_m[:], op=OP.mult)
            # broadcast each column to [128, 128] via transpose + K=1 matmul
            mats = [(A_m, A_b), (r_m, r_b)]
            if not beta_zero:
                mats.append((B_m, B_b))
            for j in range(NCC):
                cs = slice(j * 128, (j + 1) * 128)
                for mat, dst in mats:
                    prow = ptile()
                    nc.tensor.transpose(prow[0:1, 0:128], mat[:, j:j + 1],
                                        ident[:])
                    rw = small.tile([1, 128], F32, tag="rw", name="rw")
                    nc.vector.tensor_copy(rw[:], prow[0:1, 0:128])
                    pbc = ptile()
                    nc.tensor.matmul(pbc[:, 0:128], ones_row[:], rw[:],
                                     start=True, stop=True)
                    nc.vector.tensor_copy(dst[:, cs], pbc[:, 0:128])

            # ============ Pass X2 + transpose + matmul ====================
            for j in range(NCC):
                nc.vector.memset(xqt_all[:, j, XPAD - 1:XPAD], 0.0)
                nc.vector.memset(xqt_all[:, j, XPAD + T:XPAD + T + 1], 0.0)

            def produce(q):
                for itq in range(NTQ):
                    it = q * NTQ + itq
                    xt = xin2.tile([128, C], F32, tag="xt2", name="xt2")
                    nc.scalar.dma_start(out=xt[:],
                                        in_=x_d[it * 128:(it + 1) * 128, :])
                    xh = xhat_p.tile([128, C], F32, tag="xh2", name="xh2")
                    nc.scalar.activation(xh[:], xt[:], AF.Identity,
                                         bias=nmr_all[:, it:it + 1],
                                         scale=rsig_all[:, it:it + 1])
                    nc.vector.tensor_tensor(xh[:], xh[:], A_b[:], op=OP.mult)
                    if not beta_zero:
                        nc.vector.tensor_tensor(xh[:], xh[:], B_b[:], op=OP.add)
                    nc.scalar.activation(xh[:], xh[:], AF.Identity,
                                         bias=rcp_col[:], scale=1.0)
                    xq = xq_p.tile([128, C], BF16, tag="xq", name="xq")
                    nc.scalar.activation(xq[:], xh[:], AF.Identity,
                                         bias=rcn_col[:], scale=1.0)
                    nc.scalar.dma_start(out=xq_dram[it * 128:(it + 1) * 128, :],
                                        in_=xq[:])
                for j in range(NCC):
                    nc.sync.dma_start_transpose(
                        xqt_all[:, j, XPAD + q * TQ:XPAD + (q + 1) * TQ],
                        xq_dram[q * TQ:(q + 1) * TQ, j * 128:(j + 1) * 128])

            def consume(q):
                for itq in range(NTQ):
                    it = q * NTQ + itq
                    pss = [psum_mm.tile([128, OSL], F32, tag="mm", name="pmm")
                           for _ in range(NH)]
                    for j in range(NCC):
                        for k in range(KW):
                            lhsT = xqt_all[:, j, XPAD + it * 128 + k - 1:
                                           XPAD + it * 128 + k - 1 + 128]
                            first = (j == 0 and k == 0)
                            last = (j == NCC - 1 and k == KW - 1)
                            for h in range(NH):
                                nc.tensor.matmul(
                                    pss[h][:], lhsT,
                                    wqt_all[:, k, j, h * OSL:(h + 1) * OSL],
                                    start=first, stop=last)
                    for h in range(NH):
                        yt = yout.tile([128, OSL], F32, tag="yt", name="yt")
                        nc.vector.tensor_tensor(
                            yt[:], pss[h][:], r_b[:, h * OSL:(h + 1) * OSL],
                            op=OP.mult)
                        nc.gpsimd.dma_start(
                            out=out_d[it * 128:(it + 1) * 128,
                                      h * OSL:(h + 1) * OSL],
                            in_=yt[:])

            for q in range(NQ):
                produce(q)
                if q >= 1:
                    consume(q - 1)
            consume(NQ - 1)

    nc.compile()
    return nc


_NC_CACHE = {}


def _get_nc(T, C, beta_zero):
    key = (T, C, beta_zero)
    if key not in _NC_CACHE:
        _NC_CACHE[key] = build_kernel(T, C, beta_zero)
    return _NC_CACHE[key]


def run(inputs, trace=False):
    """Run the SPMD kernel; returns (output [B,T,C], BassKernelResults)."""
    x = np.ascontiguousarray(np.asarray(inputs["x"], dtype=np.float32))
    g = np.ascontiguousarray(np.asarray(inputs["ln_gamma"], dtype=np.float32))
    b = np.ascontiguousarray(np.asarray(inputs["ln_beta"], dtype=np.float32))
    W = np.ascontiguousarray(np.asarray(inputs["W"], dtype=np.float32))
    B, T, C = x.shape
    assert B == N_CORES, f"expected batch {N_CORES}, got {B}"
    beta_zero = bool(np.all(b == 0.0))
    nc = _get_nc(T, C, beta_zero)
    in_maps = [
        {"x": np.ascontiguousarray(x[i]), "ln_gamma": g, "ln_beta": b, "W": W}
        for i in range(B)
    ]
    res = run_bass_kernel_spmd(nc, in_maps, core_ids=list(range(N_CORES)),
                               trace=trace)
    out = np.stack([res.results[i]["out"] for i in range(B)], axis=0)
    return out, res


def kernel(**inputs) -> np.ndarray:
    out, _ = run(inputs)
    return out
